# revision 11
# baseline (speedup 1.0000x reference)
"""3-level 1D DWT (12-tap analysis filter bank, stride 2, pywt 'zero' mode)
for x:(16,64,16384) f32 on 8 trn2 NeuronCores.

Data-parallel over the 1024 (B,C) rows -> 128 rows/core. Signals live
position-major on chip: [partition = position%128, free = (block, row)],
built host-side (transposes are free on the host). Each level's stride-2
conv is a banded matmul; lo and hi filters are packed into one 128-wide
stationary matrix covering a 64-output chunk (64 lo rows + 64 hi rows), so
each output chunk needs only TWO accumulating matmuls (input chunks c-1, c).
Output chunks alternate even/odd parity; parity-specific W matrices place
results so every PSUM->SBUF copy is partition-aligned. lo chunks are copied
straight into the next level's input layout; hi (and lo3) go to fp16 staging
and are DMA'd out position-major, inverted on the host. Matmuls run in fp16
(fp32 PSUM accumulation); zero pad blocks reproduce pywt zero-padding.
"""

import numpy as np

import concourse.bacc as bacc
import concourse.mybir as mybir
from concourse import bass_utils
from concourse.tile import TileContext

F32 = mybir.dt.float32
F16 = mybir.dt.float16

N_CORES = 8
R = 128
L = 12
N0 = 16384

# per-level: out chunks are 64 outputs; out chunk c reads input (128-pos)
# chunks c-1, c (buffer block = chunk + 1). quad = 16 out chunks.
LEVELS = [
    dict(quads=8, pc=128),  # L1: out chunks 0..128, in = x (130 blocks)
    dict(quads=4, pc=64),   # L2: out chunks 0..64,  in = lo1 (66 blocks)
    dict(quads=2, pc=32),   # L3: out chunks 0..32,  in = lo2 (34 blocks)
]

X_BLOCKS = 130
_COMPILED = None


def _build():
    nc = bacc.Bacc(
        "TRN2",
        target_bir_lowering=False,
        debug=False,
        enable_asserts=False,
        num_devices=N_CORES,
    )
    x_in = nc.dram_tensor("x_in", [R, X_BLOCKS * 128], F16, kind="ExternalInput")
    w_in = nc.dram_tensor("w_in", [R, 4 * 128], F16, kind="ExternalInput")
    outs = {}
    outs["hi1"] = nc.dram_tensor("hi1_out", [R, 65 * 128], F16, kind="ExternalOutput")
    outs["hi2"] = nc.dram_tensor("hi2_out", [R, 33 * 128], F16, kind="ExternalOutput")
    outs["hi3"] = nc.dram_tensor("hi3_out", [R, 17 * 128], F16, kind="ExternalOutput")
    outs["lo3"] = nc.dram_tensor("lo3_out", [R, 18 * 128], F16, kind="ExternalOutput")

    with TileContext(nc) as tc:
        with (
            tc.tile_pool(name="const", bufs=1) as cpool,
            tc.tile_pool(name="xg", bufs=3) as xpool,
            tc.tile_pool(name="lobuf", bufs=1) as lpool,
            tc.tile_pool(name="comb", bufs=3) as mpool,
            tc.tile_pool(name="psum", bufs=4, space="PSUM") as ppool,
        ):
            w_sb = cpool.tile([128, 4 * 128], F16, tag="w")
            nc.sync.dma_start(w_sb[:], w_in[:])

            lo1 = lpool.tile([128, 66 * 128], F16, tag="lo1")
            lo2 = lpool.tile([128, 34 * 128], F16, tag="lo2")

            # zero pad blocks (pywt zero padding falls out of these);
            # HBM outputs are pre-zeroed so hi/lo3 pads need nothing.
            nc.gpsimd.memset(lo1[:, 0:128], 0.0)
            nc.gpsimd.memset(lo1[:, 65 * 128 : 66 * 128], 0.0)
            nc.gpsimd.memset(lo2[:, 0:128], 0.0)
            nc.gpsimd.memset(lo2[:, 33 * 128 : 34 * 128], 0.0)

            # W slices: [even_cur, even_prev, odd_cur, odd_prev]
            wEC = w_sb[:, 0:128]
            wEP = w_sb[:, 128:256]
            wOC = w_sb[:, 256:384]
            wOP = w_sb[:, 384:512]

            def quad_matmuls(lv, in_buf, m, cb, cb_off):
                """One quad (16 out chunks): 8 matmuls -> psum -> one
                full-width fp16 copy per psum tile into cb at cb_off."""
                b0 = 16 * m
                if in_buf is None:
                    src = xpool.tile([128, 18 * 128], F16, tag="xg")
                    nc.sync.dma_start(src[:, 0 : 9 * 128],
                                      x_in[:, b0 * 128 : (b0 + 9) * 128])
                    nc.sync.dma_start(src[:, 9 * 128 :],
                                      x_in[:, (b0 + 9) * 128 : (b0 + 18) * 128])
                    v = src[:]
                else:
                    v = in_buf[:, b0 * 128 : (b0 + 18) * 128]
                v4 = v.rearrange("p (t u k) -> p t u k", u=2, k=128)
                ps_e = ppool.tile([128, 1024], F32, tag="ps")
                ps_o = ppool.tile([128, 1024], F32, tag="ps")
                for h in range(2):
                    s = slice(h * 512, h * 512 + 512)
                    t = slice(h * 4, h * 4 + 4)
                    t1 = slice(h * 4 + 1, h * 4 + 5)
                    nc.tensor.matmul(ps_e[:, s], wEC, v4[:, t, 1, :],
                                     start=True, stop=False)
                    nc.tensor.matmul(ps_e[:, s], wEP, v4[:, t, 0, :],
                                     start=False, stop=True)
                    nc.tensor.matmul(ps_o[:, s], wOC, v4[:, t1, 0, :],
                                     start=True, stop=False)
                    nc.tensor.matmul(ps_o[:, s], wOP, v4[:, t, 1, :],
                                     start=False, stop=True)
                nc.vector.tensor_copy(cb[:, cb_off : cb_off + 1024], ps_e[:])
                nc.scalar.copy(cb[:, cb_off + 1024 : cb_off + 2048], ps_o[:])

            def run_level(lv, in_buf, lo_dst, lo_hbm, hi_hbm):
                cfg = LEVELS[lv]
                for pr in range(cfg["quads"] // 2):  # quad pairs
                    cb = mpool.tile([128, 4096], F16, tag="cb")
                    quad_matmuls(lv, in_buf, 2 * pr, cb, 0)
                    quad_matmuls(lv, in_buf, 2 * pr + 1, cb, 2048)
                    # cb cols: [qA evens | qA odds | qB evens | qB odds]
                    c4 = cb[:].rearrange("p (a b k) -> p a b k", b=2, k=1024)
                    lo_b = (16 * pr + 1) * 128
                    hi_b = 16 * pr * 128
                    ldst = lo_dst if lo_dst is not None else lo_hbm
                    nc.sync.dma_start(
                        ldst[0:64, lo_b : lo_b + 2048], c4[0:64, :, 0, :])
                    nc.sync.dma_start(
                        ldst[64:128, lo_b : lo_b + 2048], c4[64:128, :, 1, :])
                    hi_eng = nc.scalar if lv == 0 else nc.sync
                    hi_eng.dma_start(
                        hi_hbm[64:128, hi_b : hi_b + 2048], c4[64:128, :, 0, :])
                    hi_eng.dma_start(
                        hi_hbm[0:64, hi_b : hi_b + 2048], c4[0:64, :, 1, :])
                # partial: one even out chunk pc; in blocks pc, pc+1
                pc = cfg["pc"]
                if in_buf is None:
                    src = xpool.tile([128, 2 * 128], F16, tag="xp")
                    nc.sync.dma_start(src[:], x_in[:, pc * 128 : (pc + 2) * 128])
                    v = src[:]
                else:
                    v = in_buf[:, pc * 128 : (pc + 2) * 128]
                pp = ppool.tile([128, 1024], F32, tag="ps")
                nc.tensor.matmul(pp[:, 0:128], wEC, v[:, 128:256],
                                 start=True, stop=False)
                nc.tensor.matmul(pp[:, 0:128], wEP, v[:, 0:128],
                                 start=False, stop=True)
                cb = mpool.tile([128, 4096], F16, tag="cb")
                nc.vector.tensor_copy(cb[:, 0:128], pp[:, 0:128])
                lo_b = (pc // 2 + 1) * 128
                hi_b = (pc // 2) * 128
                ldst = lo_dst if lo_dst is not None else lo_hbm
                nc.sync.dma_start(ldst[0:64, lo_b : lo_b + 128], cb[0:64, 0:128])
                nc.sync.dma_start(hi_hbm[64:128, hi_b : hi_b + 128],
                                  cb[64:128, 0:128])

            run_level(0, None, lo1, None, outs["hi1"])
            run_level(1, lo1, lo2, None, outs["hi2"])
            run_level(2, lo2, None, outs["lo3"], outs["hi3"])

    nc.compile()
    return nc


def get_compiled():
    global _COMPILED
    if _COMPILED is None:
        _COMPILED = _build()
    return _COMPILED


def make_weights(hac: np.ndarray) -> np.ndarray:
    """Four parity/delta band matrices [p, which, j] -> [128, 512] fp16.

    even chunks: psum j<64 = lo jj=j, j>=64 = hi jj=j-64
    odd  chunks: psum j<64 = hi jj=j, j>=64 = lo jj=j-64
    cur: tap = p - 2*jj + 10 ; prev: tap = p - 118 - 2*jj
    """
    hac = np.asarray(hac, dtype=np.float32)
    sign = np.where(np.arange(L) % 2 == 0, -1.0, 1.0).astype(np.float32)
    h0 = hac
    h1 = (hac[::-1] * sign).astype(np.float32)
    W = np.zeros((128, 4, 128), dtype=np.float32)
    p = np.arange(128)[:, None]
    j = np.arange(128)
    jj = np.where(j < 64, j, j - 64)
    for wi, (lofirst, off) in enumerate(
        [(True, 10), (True, -118), (False, 10), (False, -118)]
    ):
        i = p + off - 2 * jj
        mask = (i >= 0) & (i < L)
        ic = np.clip(i, 0, L - 1)
        lo_cols = (j < 64) if lofirst else (j >= 64)
        vals = np.where(lo_cols[None, :], h0[ic], h1[ic])
        W[:, wi, :][mask] = vals[mask]
    return W.reshape(128, 512).astype(np.float16)


def make_core_input(x_rows: np.ndarray) -> np.ndarray:
    """x_rows: [128, 16384] -> position-major fp16 [128, X_BLOCKS*128].

    buffer block b holds input chunk b-1 (blocks 0 and 129 are zeros)."""
    A = np.zeros((128, X_BLOCKS, 128), dtype=np.float16)
    A[:, 1:129, :] = x_rows.reshape(128, 128, 128).transpose(2, 1, 0).astype(np.float16)
    return np.ascontiguousarray(A.reshape(128, X_BLOCKS * 128))


def unpack_std(a: np.ndarray, outsize: int, skip_blocks: int = 0) -> np.ndarray:
    """standard chunk layout [p, (blk, r)] -> [rows, outsize] fp32."""
    C = a.shape[1] // 128
    f = a.reshape(128, C, 128)[:, skip_blocks:, :]
    C -= skip_blocks
    return (
        f.transpose(2, 1, 0).reshape(128, C * 128)[:, :outsize]
    ).astype(np.float32)


def unpack_hi(a: np.ndarray, outsize: int) -> np.ndarray:
    """hi staging: partitions rotated by 64 (even chunks live at 64:128)."""
    return unpack_std(np.roll(a, -64, axis=0), outsize)


def run_cores(x: np.ndarray, hac: np.ndarray, trace: bool = False):
    x = np.asarray(x, dtype=np.float32)
    B, Ch, N = x.shape
    rows = x.reshape(B * Ch, N)
    W = make_weights(hac)
    in_maps = [
        {"x_in": make_core_input(rows[k * R : (k + 1) * R]), "w_in": W}
        for k in range(N_CORES)
    ]
    nc = get_compiled()
    res = bass_utils.run_bass_kernel_spmd(
        nc, in_maps, core_ids=list(range(N_CORES)), trace=trace
    )
    specs = [
        ("lo3_out", 2057, lambda a, s: unpack_std(a, s, skip_blocks=1)),
        ("hi1_out", 8197, unpack_hi),
        ("hi2_out", 4104, unpack_hi),
        ("hi3_out", 2057, unpack_hi),
    ]
    full = []
    for name, sz, unpack in specs:
        parts = [unpack(res.results[k][name], sz) for k in range(N_CORES)]
        full.append(np.concatenate(parts, axis=0).reshape(B, Ch, sz))
    return tuple(full), res


def kernel(x: np.ndarray, hac: np.ndarray):
    out, _ = run_cores(x, hac, trace=False)
    return out


# revision 12
# speedup vs baseline: 1.1644x; 1.1644x over previous
"""3-level 1D DWT (12-tap analysis filter bank, stride 2, pywt 'zero' mode)
for x:(16,64,16384) f32 on 8 trn2 NeuronCores.

Data-parallel over the 1024 (B,C) rows -> 128 rows/core. Signals live
position-major on chip: [partition = position%128, free = (block, row)],
built host-side (transposes are free on the host). Each level's stride-2
conv is a banded matmul; lo and hi filters are packed into one 128-wide
stationary matrix covering a 64-output chunk (64 lo rows + 64 hi rows), so
each output chunk needs only TWO accumulating matmuls (input chunks c-1, c).
Output chunks alternate even/odd parity; parity-specific W matrices place
results so every PSUM->SBUF copy is partition-aligned. lo chunks are copied
straight into the next level's input layout; hi (and lo3) go to fp16 staging
and are DMA'd out position-major, inverted on the host. Matmuls run in fp16
(fp32 PSUM accumulation); zero pad blocks reproduce pywt zero-padding.
"""

import numpy as np

import concourse.bacc as bacc
import concourse.mybir as mybir
from concourse import bass_utils
from concourse.tile import TileContext

F32 = mybir.dt.float32
F16 = mybir.dt.float16

N_CORES = 8
R = 128
L = 12
N0 = 16384

# per-level: out chunks are 64 outputs; out chunk c reads input (128-pos)
# chunks c-1, c (buffer block = chunk + 1). quad = 16 out chunks.
LEVELS = [
    dict(quads=8, pc=128),  # L1: out chunks 0..128, in = x (130 blocks)
    dict(quads=4, pc=64),   # L2: out chunks 0..64,  in = lo1 (66 blocks)
    dict(quads=2, pc=32),   # L3: out chunks 0..32,  in = lo2 (34 blocks)
]

X_BLOCKS = 130
_COMPILED = None


def _build():
    nc = bacc.Bacc(
        "TRN2",
        target_bir_lowering=False,
        debug=False,
        enable_asserts=False,
        num_devices=N_CORES,
    )
    x_in = nc.dram_tensor("x_in", [R, X_BLOCKS * 128], F16, kind="ExternalInput")
    w_in = nc.dram_tensor("w_in", [R, 4 * 128], F16, kind="ExternalInput")
    outs = {}
    outs["hi1"] = nc.dram_tensor("hi1_out", [R, 65 * 128], F16, kind="ExternalOutput")
    outs["hi2"] = nc.dram_tensor("hi2_out", [R, 33 * 128], F16, kind="ExternalOutput")
    outs["hi3"] = nc.dram_tensor("hi3_out", [R, 17 * 128], F16, kind="ExternalOutput")
    outs["lo3"] = nc.dram_tensor("lo3_out", [R, 18 * 128], F16, kind="ExternalOutput")

    with TileContext(nc) as tc:
        with (
            tc.tile_pool(name="const", bufs=1) as cpool,
            tc.tile_pool(name="xg", bufs=3) as xpool,
            tc.tile_pool(name="lobuf", bufs=1) as lpool,
            tc.tile_pool(name="stage", bufs=1) as spool,
            tc.tile_pool(name="psum", bufs=4, space="PSUM") as ppool,
        ):
            w_sb = cpool.tile([128, 4 * 128], F16, tag="w")
            nc.sync.dma_start(w_sb[:], w_in[:])

            lo1 = lpool.tile([128, 66 * 128], F16, tag="lo1")
            lo2 = lpool.tile([128, 34 * 128], F16, tag="lo2")
            hi1s = spool.tile([128, 65 * 128], F16, tag="hi1s")
            hi2s = spool.tile([128, 33 * 128], F16, tag="hi2s")
            hi3s = spool.tile([128, 17 * 128], F16, tag="hi3s")
            lo3s = spool.tile([128, 18 * 128], F16, tag="lo3s")

            # zero pad blocks (pywt zero padding falls out of these)
            nc.gpsimd.memset(lo1[:, 0:128], 0.0)
            nc.gpsimd.memset(lo1[:, 65 * 128 : 66 * 128], 0.0)
            nc.gpsimd.memset(lo2[:, 0:128], 0.0)
            nc.gpsimd.memset(lo2[:, 33 * 128 : 34 * 128], 0.0)
            nc.gpsimd.memset(hi1s[:, 64 * 128 : 65 * 128], 0.0)
            nc.gpsimd.memset(hi2s[:, 32 * 128 : 33 * 128], 0.0)
            nc.gpsimd.memset(hi3s[:, 16 * 128 : 17 * 128], 0.0)
            nc.gpsimd.memset(lo3s[:, 0:128], 0.0)
            nc.gpsimd.memset(lo3s[:, 17 * 128 : 18 * 128], 0.0)

            # W slices: [even_cur, even_prev, odd_cur, odd_prev]
            wEC = w_sb[:, 0:128]
            wEP = w_sb[:, 128:256]
            wOC = w_sb[:, 256:384]
            wOP = w_sb[:, 384:512]

            def run_level(lv, in_buf, lo_dst, hi_dst):
                cfg = LEVELS[lv]
                for m in range(cfg["quads"]):
                    b0 = 16 * m  # window = buffer blocks b0 .. b0+17
                    if in_buf is None:
                        src = xpool.tile([128, 18 * 128], F16, tag="xg")
                        nc.sync.dma_start(
                            src[:, 0 : 9 * 128],
                            x_in[:, b0 * 128 : (b0 + 9) * 128],
                        )
                        nc.sync.dma_start(
                            src[:, 9 * 128 :],
                            x_in[:, (b0 + 9) * 128 : (b0 + 18) * 128],
                        )
                        v = src[:]
                    else:
                        v = in_buf[:, b0 * 128 : (b0 + 18) * 128]
                    # v4[:, t, u] = window block 2t+u = input chunk 16m-1+2t+u
                    v4 = v.rearrange("p (t u k) -> p t u k", u=2, k=128)
                    # evens psum: out chunks 16m+2q; odds: 16m+2q+1 (q=0..7)
                    ps_e = ppool.tile([128, 1024], F32, tag="ps")
                    ps_o = ppool.tile([128, 1024], F32, tag="ps")
                    for h in range(2):  # bank halves, N=512 each
                        s = slice(h * 512, h * 512 + 512)
                        t = slice(h * 4, h * 4 + 4)
                        t1 = slice(h * 4 + 1, h * 4 + 5)
                        nc.tensor.matmul(ps_e[:, s], wEC, v4[:, t, 1, :],
                                         start=True, stop=False)
                        nc.tensor.matmul(ps_e[:, s], wEP, v4[:, t, 0, :],
                                         start=False, stop=True)
                        nc.tensor.matmul(ps_o[:, s], wOC, v4[:, t1, 0, :],
                                         start=True, stop=False)
                        nc.tensor.matmul(ps_o[:, s], wOP, v4[:, t, 1, :],
                                         start=False, stop=True)
                    # copies (all partition-aligned)
                    lo_b = (8 * m + 1) * 128
                    hi_b = 8 * m * 128
                    nc.vector.tensor_copy(lo_dst[0:64, lo_b : lo_b + 1024],
                                          ps_e[0:64, :])
                    nc.scalar.copy(hi_dst[64:128, hi_b : hi_b + 1024],
                                   ps_e[64:128, :])
                    nc.scalar.copy(lo_dst[64:128, lo_b : lo_b + 1024],
                                   ps_o[64:128, :])
                    nc.vector.tensor_copy(hi_dst[0:64, hi_b : hi_b + 1024],
                                          ps_o[0:64, :])
                # partial: one even out chunk pc; in blocks pc, pc+1
                pc = cfg["pc"]
                if in_buf is None:
                    src = xpool.tile([128, 2 * 128], F16, tag="xp")
                    nc.sync.dma_start(src[:], x_in[:, pc * 128 : (pc + 2) * 128])
                    v = src[:]
                else:
                    v = in_buf[:, pc * 128 : (pc + 2) * 128]
                pp = ppool.tile([128, 1024], F32, tag="ps")
                nc.tensor.matmul(pp[:, 0:128], wEC, v[:, 128:256],
                                 start=True, stop=False)
                nc.tensor.matmul(pp[:, 0:128], wEP, v[:, 0:128],
                                 start=False, stop=True)
                lo_b = (pc // 2 + 1) * 128
                hi_b = (pc // 2) * 128
                nc.vector.tensor_copy(lo_dst[0:64, lo_b : lo_b + 128],
                                      pp[0:64, 0:128])
                nc.scalar.copy(hi_dst[64:128, hi_b : hi_b + 128],
                               pp[64:128, 0:128])

            run_level(0, None, lo1, hi1s)
            run_level(1, lo1, lo2, hi2s)
            run_level(2, lo2, lo3s, hi3s)

            # finely-split output DMAs on both HWDGE rings so the out
            # stream drains continuously while compute proceeds
            for a, b in [(0, 16), (16, 32), (32, 48), (48, 65)]:
                nc.scalar.dma_start(outs["hi1"][:, a * 128 : b * 128],
                                    hi1s[:, a * 128 : b * 128])
            for a, b in [(0, 8), (8, 16), (16, 24), (24, 33)]:
                nc.sync.dma_start(outs["hi2"][:, a * 128 : b * 128],
                                  hi2s[:, a * 128 : b * 128])
            for a, b in [(0, 8), (8, 17)]:
                nc.scalar.dma_start(outs["hi3"][:, a * 128 : b * 128],
                                    hi3s[:, a * 128 : b * 128])
            for a, b in [(0, 9), (9, 18)]:
                nc.sync.dma_start(outs["lo3"][:, a * 128 : b * 128],
                                  lo3s[:, a * 128 : b * 128])

    nc.compile()
    return nc


def get_compiled():
    global _COMPILED
    if _COMPILED is None:
        _COMPILED = _build()
    return _COMPILED


def make_weights(hac: np.ndarray) -> np.ndarray:
    """Four parity/delta band matrices [p, which, j] -> [128, 512] fp16.

    even chunks: psum j<64 = lo jj=j, j>=64 = hi jj=j-64
    odd  chunks: psum j<64 = hi jj=j, j>=64 = lo jj=j-64
    cur: tap = p - 2*jj + 10 ; prev: tap = p - 118 - 2*jj
    """
    hac = np.asarray(hac, dtype=np.float32)
    sign = np.where(np.arange(L) % 2 == 0, -1.0, 1.0).astype(np.float32)
    h0 = hac
    h1 = (hac[::-1] * sign).astype(np.float32)
    W = np.zeros((128, 4, 128), dtype=np.float32)
    p = np.arange(128)[:, None]
    j = np.arange(128)
    jj = np.where(j < 64, j, j - 64)
    for wi, (lofirst, off) in enumerate(
        [(True, 10), (True, -118), (False, 10), (False, -118)]
    ):
        i = p + off - 2 * jj
        mask = (i >= 0) & (i < L)
        ic = np.clip(i, 0, L - 1)
        lo_cols = (j < 64) if lofirst else (j >= 64)
        vals = np.where(lo_cols[None, :], h0[ic], h1[ic])
        W[:, wi, :][mask] = vals[mask]
    return W.reshape(128, 512).astype(np.float16)


def make_core_input(x_rows: np.ndarray) -> np.ndarray:
    """x_rows: [128, 16384] -> position-major fp16 [128, X_BLOCKS*128].

    buffer block b holds input chunk b-1 (blocks 0 and 129 are zeros)."""
    A = np.zeros((128, X_BLOCKS, 128), dtype=np.float16)
    A[:, 1:129, :] = x_rows.reshape(128, 128, 128).transpose(2, 1, 0).astype(np.float16)
    return np.ascontiguousarray(A.reshape(128, X_BLOCKS * 128))


def unpack_std(a: np.ndarray, outsize: int, skip_blocks: int = 0) -> np.ndarray:
    """standard chunk layout [p, (blk, r)] -> [rows, outsize] fp32."""
    C = a.shape[1] // 128
    f = a.reshape(128, C, 128)[:, skip_blocks:, :]
    C -= skip_blocks
    return (
        f.transpose(2, 1, 0).reshape(128, C * 128)[:, :outsize]
    ).astype(np.float32)


def unpack_hi(a: np.ndarray, outsize: int) -> np.ndarray:
    """hi staging: partitions rotated by 64 (even chunks live at 64:128)."""
    return unpack_std(np.roll(a, -64, axis=0), outsize)


def run_cores(x: np.ndarray, hac: np.ndarray, trace: bool = False):
    x = np.asarray(x, dtype=np.float32)
    B, Ch, N = x.shape
    rows = x.reshape(B * Ch, N)
    W = make_weights(hac)
    in_maps = [
        {"x_in": make_core_input(rows[k * R : (k + 1) * R]), "w_in": W}
        for k in range(N_CORES)
    ]
    nc = get_compiled()
    res = bass_utils.run_bass_kernel_spmd(
        nc, in_maps, core_ids=list(range(N_CORES)), trace=trace
    )
    specs = [
        ("lo3_out", 2057, lambda a, s: unpack_std(a, s, skip_blocks=1)),
        ("hi1_out", 8197, unpack_hi),
        ("hi2_out", 4104, unpack_hi),
        ("hi3_out", 2057, unpack_hi),
    ]
    full = []
    for name, sz, unpack in specs:
        parts = [unpack(res.results[k][name], sz) for k in range(N_CORES)]
        full.append(np.concatenate(parts, axis=0).reshape(B, Ch, sz))
    return tuple(full), res


def kernel(x: np.ndarray, hac: np.ndarray):
    out, _ = run_cores(x, hac, trace=False)
    return out


# revision 14
# speedup vs baseline: 1.2478x; 1.0716x over previous
"""3-level 1D DWT (12-tap analysis filter bank, stride 2, pywt 'zero' mode)
for x:(16,64,16384) f32 on 8 trn2 NeuronCores.

Data-parallel over the 1024 (B,C) rows -> 128 rows/core. Signals live
position-major on chip: [partition = position%128, free = (block, row)],
built host-side (transposes are free on the host). Each level's stride-2
conv is a banded matmul; lo and hi filters are packed into one 128-wide
stationary matrix covering a 64-output chunk (64 lo rows + 64 hi rows), so
each output chunk needs only TWO accumulating matmuls (input chunks c-1, c).
Output chunks alternate even/odd parity; parity-specific W matrices place
results so every PSUM->SBUF copy is partition-aligned. lo chunks are copied
straight into the next level's input layout; hi (and lo3) go to fp16 staging
and are DMA'd out position-major, inverted on the host. Matmuls run in fp16
(fp32 PSUM accumulation); zero pad blocks reproduce pywt zero-padding.
"""

import numpy as np

import concourse.bacc as bacc
import concourse.mybir as mybir
from concourse import bass_utils
from concourse.tile import TileContext

F32 = mybir.dt.float32
F16 = mybir.dt.float16

N_CORES = 8
R = 128
L = 12
N0 = 16384

# per-level: out chunks are 64 outputs; out chunk c reads input (128-pos)
# chunks c-1, c (buffer block = chunk + 1). quad = 16 out chunks.
LEVELS = [
    dict(quads=8, pc=128),  # L1: out chunks 0..128, in = x (130 blocks)
    dict(quads=4, pc=64),   # L2: out chunks 0..64,  in = lo1 (66 blocks)
    dict(quads=2, pc=32),   # L3: out chunks 0..32,  in = lo2 (34 blocks)
]

X_BLOCKS = 130
_COMPILED = None


def _build():
    nc = bacc.Bacc(
        "TRN2",
        target_bir_lowering=False,
        debug=False,
        enable_asserts=False,
        num_devices=N_CORES,
    )
    x_in = nc.dram_tensor("x_in", [R, X_BLOCKS * 128], F16, kind="ExternalInput")
    w_in = nc.dram_tensor("w_in", [R, 4 * 128], F16, kind="ExternalInput")
    outs = {}
    outs["hi1"] = nc.dram_tensor("hi1_out", [R, 65 * 128], F16, kind="ExternalOutput")
    outs["hi2"] = nc.dram_tensor("hi2_out", [R, 33 * 128], F16, kind="ExternalOutput")
    outs["hi3"] = nc.dram_tensor("hi3_out", [R, 17 * 128], F16, kind="ExternalOutput")
    outs["lo3"] = nc.dram_tensor("lo3_out", [R, 18 * 128], F16, kind="ExternalOutput")

    with TileContext(nc) as tc:
        with (
            tc.tile_pool(name="const", bufs=1) as cpool,
            tc.tile_pool(name="xg", bufs=3) as xpool,
            tc.tile_pool(name="lobuf", bufs=1) as lpool,
            tc.tile_pool(name="stage", bufs=1) as spool,
            tc.tile_pool(name="psum", bufs=4, space="PSUM") as ppool,
        ):
            w_sb = cpool.tile([128, 4 * 128], F16, tag="w")
            nc.sync.dma_start(w_sb[:], w_in[:])

            lo1 = lpool.tile([128, 66 * 128], F16, tag="lo1")
            lo2 = lpool.tile([128, 34 * 128], F16, tag="lo2")
            hi1s = spool.tile([128, 65 * 128], F16, tag="hi1s")
            hi2s = spool.tile([128, 33 * 128], F16, tag="hi2s")
            hi3s = spool.tile([128, 17 * 128], F16, tag="hi3s")
            lo3s = spool.tile([128, 18 * 128], F16, tag="lo3s")

            # zero pad blocks (pywt zero padding falls out of these)
            nc.gpsimd.memset(lo1[:, 0:128], 0.0)
            nc.gpsimd.memset(lo1[:, 65 * 128 : 66 * 128], 0.0)
            nc.gpsimd.memset(lo2[:, 0:128], 0.0)
            nc.gpsimd.memset(lo2[:, 33 * 128 : 34 * 128], 0.0)
            nc.gpsimd.memset(hi1s[:, 64 * 128 : 65 * 128], 0.0)
            nc.gpsimd.memset(hi2s[:, 32 * 128 : 33 * 128], 0.0)
            nc.gpsimd.memset(hi3s[:, 16 * 128 : 17 * 128], 0.0)
            nc.gpsimd.memset(lo3s[:, 0:128], 0.0)
            nc.gpsimd.memset(lo3s[:, 17 * 128 : 18 * 128], 0.0)

            # W slices: [even_cur, even_prev, odd_cur, odd_prev]
            wEC = w_sb[:, 0:128]
            wEP = w_sb[:, 128:256]
            wOC = w_sb[:, 256:384]
            wOP = w_sb[:, 384:512]

            # per-level (input buffer, lo dst, hi dst); None input = x_in
            LV = [
                (None, lo1, hi1s),
                (lo1, lo2, hi2s),
                (lo2, lo3s, hi3s),
            ]

            def emit_quad(lv, m):
                in_buf, lo_dst, hi_dst = LV[lv]
                b0 = 16 * m  # window = buffer blocks b0 .. b0+17
                if in_buf is None:
                    src = xpool.tile([128, 18 * 128], F16, tag="xg")
                    nc.sync.dma_start(src[:, 0 : 9 * 128],
                                      x_in[:, b0 * 128 : (b0 + 9) * 128])
                    nc.sync.dma_start(src[:, 9 * 128 :],
                                      x_in[:, (b0 + 9) * 128 : (b0 + 18) * 128])
                    v = src[:]
                else:
                    v = in_buf[:, b0 * 128 : (b0 + 18) * 128]
                # v4[:, t, u] = window block 2t+u = input chunk 16m-1+2t+u
                v4 = v.rearrange("p (t u k) -> p t u k", u=2, k=128)
                # evens psum: out chunks 16m+2q; odds: 16m+2q+1 (q=0..7)
                ps_e = ppool.tile([128, 1024], F32, tag="ps")
                ps_o = ppool.tile([128, 1024], F32, tag="ps")
                for h in range(2):  # bank halves, N=512 each
                    s = slice(h * 512, h * 512 + 512)
                    t = slice(h * 4, h * 4 + 4)
                    t1 = slice(h * 4 + 1, h * 4 + 5)
                    nc.tensor.matmul(ps_e[:, s], wEC, v4[:, t, 1, :],
                                     start=True, stop=False)
                    nc.tensor.matmul(ps_e[:, s], wEP, v4[:, t, 0, :],
                                     start=False, stop=True)
                    nc.tensor.matmul(ps_o[:, s], wOC, v4[:, t1, 0, :],
                                     start=True, stop=False)
                    nc.tensor.matmul(ps_o[:, s], wOP, v4[:, t, 1, :],
                                     start=False, stop=True)
                # copies (all partition-aligned)
                lo_b = (8 * m + 1) * 128
                hi_b = 8 * m * 128
                nc.vector.tensor_copy(lo_dst[0:64, lo_b : lo_b + 1024],
                                      ps_e[0:64, :])
                nc.scalar.copy(hi_dst[64:128, hi_b : hi_b + 1024],
                               ps_e[64:128, :])
                nc.scalar.copy(lo_dst[64:128, lo_b : lo_b + 1024],
                               ps_o[64:128, :])
                nc.vector.tensor_copy(hi_dst[0:64, hi_b : hi_b + 1024],
                                      ps_o[0:64, :])

            def emit_partial(lv):
                in_buf, lo_dst, hi_dst = LV[lv]
                pc = LEVELS[lv]["pc"]
                if in_buf is None:
                    src = xpool.tile([128, 2 * 128], F16, tag="xp")
                    nc.sync.dma_start(src[:], x_in[:, pc * 128 : (pc + 2) * 128])
                    v = src[:]
                else:
                    v = in_buf[:, pc * 128 : (pc + 2) * 128]
                pp = ppool.tile([128, 1024], F32, tag="ps")
                nc.tensor.matmul(pp[:, 0:128], wEC, v[:, 128:256],
                                 start=True, stop=False)
                nc.tensor.matmul(pp[:, 0:128], wEP, v[:, 0:128],
                                 start=False, stop=True)
                lo_b = (pc // 2 + 1) * 128
                hi_b = (pc // 2) * 128
                nc.vector.tensor_copy(lo_dst[0:64, lo_b : lo_b + 128],
                                      pp[0:64, 0:128])
                nc.scalar.copy(hi_dst[64:128, hi_b : hi_b + 128],
                               pp[64:128, 0:128])

            # interleaved emission: each L2/L3 quad enters the in-order PE
            # stream at its earliest dependency-safe point (L_n quad g needs
            # L_{n-1} quads <= 2g+2), so level transitions don't stall on
            # lagging copies and PSUM-slot pressure stays smooth.
            schedule = [
                (0, 0), (0, 1), (0, 2), (0, 3),
                (1, 0),
                (0, 4),
                (1, 1),
                (0, 5), (0, 6),
                (1, 2),
                (0, 7), (0, "p"),
                (1, 3),
                (2, 0),
                (1, "p"),
                (2, 1), (2, "p"),
            ]
            for lv, m in schedule:
                if m == "p":
                    emit_partial(lv)
                else:
                    emit_quad(lv, m)

            # finely-split output DMAs on both HWDGE rings so the out
            # stream drains continuously while compute proceeds
            for a, b in [(0, 16), (16, 32), (32, 48), (48, 65)]:
                nc.sync.dma_start(outs["hi1"][:, a * 128 : b * 128],
                                  hi1s[:, a * 128 : b * 128])
            for a, b in [(0, 8), (8, 16), (16, 24), (24, 33)]:
                nc.sync.dma_start(outs["hi2"][:, a * 128 : b * 128],
                                  hi2s[:, a * 128 : b * 128])
            for a, b in [(0, 8), (8, 17)]:
                nc.sync.dma_start(outs["hi3"][:, a * 128 : b * 128],
                                  hi3s[:, a * 128 : b * 128])
            for a, b in [(0, 9), (9, 18)]:
                nc.sync.dma_start(outs["lo3"][:, a * 128 : b * 128],
                                  lo3s[:, a * 128 : b * 128])

    nc.compile()
    return nc


def get_compiled():
    global _COMPILED
    if _COMPILED is None:
        _COMPILED = _build()
    return _COMPILED


def make_weights(hac: np.ndarray) -> np.ndarray:
    """Four parity/delta band matrices [p, which, j] -> [128, 512] fp16.

    even chunks: psum j<64 = lo jj=j, j>=64 = hi jj=j-64
    odd  chunks: psum j<64 = hi jj=j, j>=64 = lo jj=j-64
    cur: tap = p - 2*jj + 10 ; prev: tap = p - 118 - 2*jj
    """
    hac = np.asarray(hac, dtype=np.float32)
    sign = np.where(np.arange(L) % 2 == 0, -1.0, 1.0).astype(np.float32)
    h0 = hac
    h1 = (hac[::-1] * sign).astype(np.float32)
    W = np.zeros((128, 4, 128), dtype=np.float32)
    p = np.arange(128)[:, None]
    j = np.arange(128)
    jj = np.where(j < 64, j, j - 64)
    for wi, (lofirst, off) in enumerate(
        [(True, 10), (True, -118), (False, 10), (False, -118)]
    ):
        i = p + off - 2 * jj
        mask = (i >= 0) & (i < L)
        ic = np.clip(i, 0, L - 1)
        lo_cols = (j < 64) if lofirst else (j >= 64)
        vals = np.where(lo_cols[None, :], h0[ic], h1[ic])
        W[:, wi, :][mask] = vals[mask]
    return W.reshape(128, 512).astype(np.float16)


def make_core_input(x_rows: np.ndarray) -> np.ndarray:
    """x_rows: [128, 16384] -> position-major fp16 [128, X_BLOCKS*128].

    buffer block b holds input chunk b-1 (blocks 0 and 129 are zeros)."""
    A = np.zeros((128, X_BLOCKS, 128), dtype=np.float16)
    A[:, 1:129, :] = x_rows.reshape(128, 128, 128).transpose(2, 1, 0).astype(np.float16)
    return np.ascontiguousarray(A.reshape(128, X_BLOCKS * 128))


def unpack_std(a: np.ndarray, outsize: int, skip_blocks: int = 0) -> np.ndarray:
    """standard chunk layout [p, (blk, r)] -> [rows, outsize] fp32."""
    C = a.shape[1] // 128
    f = a.reshape(128, C, 128)[:, skip_blocks:, :]
    C -= skip_blocks
    return (
        f.transpose(2, 1, 0).reshape(128, C * 128)[:, :outsize]
    ).astype(np.float32)


def unpack_hi(a: np.ndarray, outsize: int) -> np.ndarray:
    """hi staging: partitions rotated by 64 (even chunks live at 64:128)."""
    return unpack_std(np.roll(a, -64, axis=0), outsize)


def run_cores(x: np.ndarray, hac: np.ndarray, trace: bool = False):
    x = np.asarray(x, dtype=np.float32)
    B, Ch, N = x.shape
    rows = x.reshape(B * Ch, N)
    W = make_weights(hac)
    in_maps = [
        {"x_in": make_core_input(rows[k * R : (k + 1) * R]), "w_in": W}
        for k in range(N_CORES)
    ]
    nc = get_compiled()
    res = bass_utils.run_bass_kernel_spmd(
        nc, in_maps, core_ids=list(range(N_CORES)), trace=trace
    )
    specs = [
        ("lo3_out", 2057, lambda a, s: unpack_std(a, s, skip_blocks=1)),
        ("hi1_out", 8197, unpack_hi),
        ("hi2_out", 4104, unpack_hi),
        ("hi3_out", 2057, unpack_hi),
    ]
    full = []
    for name, sz, unpack in specs:
        parts = [unpack(res.results[k][name], sz) for k in range(N_CORES)]
        full.append(np.concatenate(parts, axis=0).reshape(B, Ch, sz))
    return tuple(full), res


def kernel(x: np.ndarray, hac: np.ndarray):
    out, _ = run_cores(x, hac, trace=False)
    return out
